# revision 5
# baseline (speedup 1.0000x reference)
"""Trainium2 Bass kernel for a 2-layer GRU decoder with output feedback.

Math per step t (B batch, H=512 hidden, PyTorch GRU cell semantics):
  x      = [ctx, prev]                        (B, H+1)
  h0'    = GRUCell(x, h0;  W_ih0, W_hh0, b_ih0, b_hh0)
  h1'    = GRUCell(h0', h1; W_ih1, W_hh1, b_ih1, b_hh1)
  y      = relu(W_o1 @ h1' + b_o1)
  out(t) = relu(W_o2 @ y + b_o2)              (B, 1)
  prev   = out(t)
Sequential over T=1024 steps.  Sharding: data-parallel over batch,
8 cores x B_CORE=8.  Everything lives in SBUF; activations are kept
TRANSPOSED ([H, B] with hidden on partitions) so elementwise gate math
runs on 128-partition tiles; matmuls are weight-stationary (lhsT =
weight tiles, rhs = transposed activations, N=B).  Weights in bf16
(FWL fast weight load), accumulation and gate math in fp32.
ctx contribution C0 = ctx @ W_ih0[:, :H].T (+ biases) is precomputed on
host; the prev feedback enters as a rank-1 DVE op.
"""

import sys

for _p in ("/opt/trn_rl_repo", "/opt/pypackages"):
    if _p not in sys.path:
        sys.path.insert(0, _p)

import numpy as np
import ml_dtypes  # noqa: F401  (bf16 numpy dtype)

import concourse.bass as bass
import concourse.mybir as mybir
import concourse.tile as tile
from concourse import bacc
from concourse.bass import ds
from concourse.bass_utils import run_bass_kernel_spmd

B = 64          # full batch
NCORES = 8
BC = B // NCORES  # batch per core = 8
H = 512
T = 1024
KH = H // 128   # 4 k-tiles per H
M3 = 3 * H // 128  # 12 m-tiles for gates

F32 = mybir.dt.float32
BF16 = mybir.dt.bfloat16
AF = mybir.ActivationFunctionType
OP = mybir.AluOpType


def build_nc(n_steps=T, enable_asserts=False):
    nc = bacc.Bacc("TRN2", target_bir_lowering=False, debug=False,
                   enable_asserts=enable_asserts)

    # ---- DRAM I/O (per core) ----
    d_w0t = nc.dram_tensor("w0t", [128, KH * 1536], BF16, kind="ExternalInput")
    d_w1t = nc.dram_tensor("w1t", [128, 2 * KH * 1536], BF16, kind="ExternalInput")
    d_wot = nc.dram_tensor("wot", [128, KH * 512], BF16, kind="ExternalInput")
    d_wo2c = nc.dram_tensor("wo2c", [128, KH], BF16, kind="ExternalInput")
    d_wo2r = nc.dram_tensor("wo2r", [128, KH * 128], BF16, kind="ExternalInput")
    d_c0t = nc.dram_tensor("c0t", [128, 96], F32, kind="ExternalInput")
    d_wpt = nc.dram_tensor("wpt", [128, 96], F32, kind="ExternalInput")
    d_bhn0 = nc.dram_tensor("bhn0", [128, 32], F32, kind="ExternalInput")
    d_b1rz = nc.dram_tensor("b1rz", [128, 64], F32, kind="ExternalInput")
    d_bin1 = nc.dram_tensor("bin1", [128, 32], F32, kind="ExternalInput")
    d_bhn1 = nc.dram_tensor("bhn1", [128, 32], F32, kind="ExternalInput")
    d_bo1 = nc.dram_tensor("bo1", [128, 32], F32, kind="ExternalInput")
    d_bo2r = nc.dram_tensor("bo2r", [128, 1], F32, kind="ExternalInput")
    d_oy = nc.dram_tensor("oy", [BC, n_steps], F32, kind="ExternalOutput")
    d_oh = nc.dram_tensor("oh", [128, 64], F32, kind="ExternalOutput")

    with tile.TileContext(nc) as tc:
        with tc.tile_pool(name="const", bufs=1) as cpool, \
             tc.tile_pool(name="state", bufs=1) as spool, \
             tc.tile_pool(name="scratch", bufs=2) as wpool, \
             tc.tile_pool(name="psum", bufs=1, space="PSUM") as ppool:

            # ---- persistent SBUF tiles ----
            w0t = cpool.tile([128, KH, 1536], BF16)
            w1t = cpool.tile([128, 2 * KH, 1536], BF16)
            wot = cpool.tile([128, KH, 512], BF16)
            wo2c = cpool.tile([128, KH], BF16)
            wo2r = cpool.tile([128, KH, 128], BF16)
            c0t = cpool.tile([128, 96], F32)
            wpt = cpool.tile([128, 96], F32)
            bhn0 = cpool.tile([128, 32], F32)
            b1rz = cpool.tile([128, 64], F32)
            bin1 = cpool.tile([128, 32], F32)
            bhn1 = cpool.tile([128, 32], F32)
            bo1 = cpool.tile([128, 32], F32)
            bo2r = cpool.tile([128, 1], F32)

            h0t = spool.tile([128, KH, BC], F32)
            h1t = spool.tile([128, KH, BC], F32)
            h0b = spool.tile([128, KH, BC], BF16)
            h1b = spool.tile([128, KH, BC], BF16)
            prevr = spool.tile([128, BC], F32)
            outsb = spool.tile([BC, n_steps], F32)

            # ---- load constants ----
            for sb, dr in [(w0t, d_w0t), (w1t, d_w1t), (wot, d_wot),
                           (wo2c, d_wo2c), (wo2r, d_wo2r), (c0t, d_c0t),
                           (wpt, d_wpt), (bhn0, d_bhn0), (b1rz, d_b1rz),
                           (bin1, d_bin1), (bhn1, d_bhn1), (bo1, d_bo1),
                           (bo2r, d_bo2r)]:
                nc.sync.dma_start(out=sb[:].rearrange("p ... -> p (...)"),
                                  in_=dr[:, :])

            # ---- init state ----
            for t_ in (h0t, h1t, h0b, h1b, prevr):
                nc.vector.memset(t_[:], 0)

            # ---- PSUM tiles (8 banks) ----
            p0rz = ppool.tile([128, 64], F32)
            p0hn = ppool.tile([128, 32], F32)
            p1rz = ppool.tile([128, 64], F32)
            p1in = ppool.tile([128, 32], F32)
            p1hn = ppool.tile([128, 32], F32)
            py = ppool.tile([128, 32], F32)
            poa = ppool.tile([BC, 1], F32)
            pob = ppool.tile([128, BC], F32)

            def step(iv):
                V, S = nc.vector, nc.scalar
                # --- layer-0 recurrent matmuls (need h0b, from t-1) ---
                for m in range(8):
                    for k in range(KH):
                        nc.tensor.matmul(p0rz[:, m * BC:(m + 1) * BC],
                                         w0t[:, k, m * 128:(m + 1) * 128],
                                         h0b[:, k, :],
                                         start=(m == 0 and k == 0),
                                         stop=(m == 7 and k == KH - 1))
                for m in range(4):
                    for k in range(KH):
                        nc.tensor.matmul(p0hn[:, m * BC:(m + 1) * BC],
                                         w0t[:, k, (8 + m) * 128:(9 + m) * 128],
                                         h0b[:, k, :],
                                         start=(m == 0 and k == 0),
                                         stop=(m == 3 and k == KH - 1))
                # --- layer-1 h1-side matmuls (need only h1b from t-1) ---
                for m in range(8):
                    for k in (4, 5, 6, 7):
                        nc.tensor.matmul(p1rz[:, m * BC:(m + 1) * BC],
                                         w1t[:, k, m * 128:(m + 1) * 128],
                                         h1b[:, k - 4, :],
                                         start=(m == 0 and k == 4), stop=False)
                for m in range(4):
                    for k in (4, 5, 6, 7):
                        nc.tensor.matmul(p1hn[:, m * BC:(m + 1) * BC],
                                         w1t[:, k, (8 + m) * 128:(9 + m) * 128],
                                         h1b[:, k - 4, :],
                                         start=(m == 0 and k == 4),
                                         stop=(m == 3 and k == 7))

                # --- layer-0 gates ---
                g = wpool.tile([128, 96], F32, tag="g")
                prev3 = prevr[:].rearrange("p (o b) -> p o b", o=1) \
                    .broadcast_to([128, 12, BC])
                V.tensor_tensor(g[:].rearrange("p (m b) -> p m b", m=12),
                                wpt[:].rearrange("p (m b) -> p m b", m=12),
                                prev3, OP.mult)
                prz = wpool.tile([128, 64], F32, tag="prz")
                V.tensor_tensor(prz[:], p0rz[:], c0t[:, 0:64], OP.add)
                prz2 = wpool.tile([128, 64], F32, tag="prz2")
                V.tensor_tensor(prz2[:], prz[:], g[:, 0:64], OP.add)
                rz0 = wpool.tile([128, 64], F32, tag="rz0")
                S.activation(rz0[:], prz2[:], AF.Sigmoid)
                hn0 = wpool.tile([128, 32], F32, tag="hn0")
                V.tensor_tensor(hn0[:], p0hn[:], bhn0[:], OP.add)
                d0 = wpool.tile([128, 32], F32, tag="d0")
                V.tensor_tensor(d0[:], rz0[:, 0:32], hn0[:], OP.mult)
                gi0 = wpool.tile([128, 32], F32, tag="gi0")
                V.tensor_tensor(gi0[:], g[:, 64:96], c0t[:, 64:96], OP.add)
                v0 = wpool.tile([128, 32], F32, tag="v0")
                V.tensor_tensor(v0[:], d0[:], gi0[:], OP.add)
                n0 = wpool.tile([128, 32], F32, tag="n0")
                S.activation(n0[:], v0[:], AF.Tanh)
                t20 = wpool.tile([128, 32], F32, tag="t20")
                V.tensor_tensor(t20[:], h0t[:].rearrange("p k b -> p (k b)"),
                                n0[:], OP.subtract)
                t30 = wpool.tile([128, 32], F32, tag="t30")
                V.tensor_tensor(t30[:], rz0[:, 32:64], t20[:], OP.mult)
                V.tensor_tensor(h0t[:].rearrange("p k b -> p (k b)"),
                                t30[:], n0[:], OP.add)
                S.activation(h0b[:].rearrange("p k b -> p (k b)"),
                             h0t[:].rearrange("p k b -> p (k b)"), AF.Copy)

                # --- layer-1 h0'-side matmuls ---
                for m in range(8):
                    for k in range(4):
                        nc.tensor.matmul(p1rz[:, m * BC:(m + 1) * BC],
                                         w1t[:, k, m * 128:(m + 1) * 128],
                                         h0b[:, k, :],
                                         start=False, stop=(m == 7 and k == 3))
                for m in range(4):
                    for k in range(4):
                        nc.tensor.matmul(p1in[:, m * BC:(m + 1) * BC],
                                         w1t[:, k, (8 + m) * 128:(9 + m) * 128],
                                         h0b[:, k, :],
                                         start=(m == 0 and k == 0),
                                         stop=(m == 3 and k == 3))

                # --- layer-1 gates ---
                q1 = wpool.tile([128, 64], F32, tag="q1")
                V.tensor_tensor(q1[:], p1rz[:], b1rz[:], OP.add)
                rz1 = wpool.tile([128, 64], F32, tag="rz1")
                S.activation(rz1[:], q1[:], AF.Sigmoid)
                c1 = wpool.tile([128, 32], F32, tag="c1")
                V.tensor_tensor(c1[:], p1hn[:], bhn1[:], OP.add)
                e1 = wpool.tile([128, 32], F32, tag="e1")
                V.tensor_tensor(e1[:], p1in[:], bin1[:], OP.add)
                d1 = wpool.tile([128, 32], F32, tag="d1")
                V.tensor_tensor(d1[:], rz1[:, 0:32], c1[:], OP.mult)
                v1 = wpool.tile([128, 32], F32, tag="v1")
                V.tensor_tensor(v1[:], d1[:], e1[:], OP.add)
                n1 = wpool.tile([128, 32], F32, tag="n1")
                S.activation(n1[:], v1[:], AF.Tanh)
                t21 = wpool.tile([128, 32], F32, tag="t21")
                V.tensor_tensor(t21[:], h1t[:].rearrange("p k b -> p (k b)"),
                                n1[:], OP.subtract)
                t31 = wpool.tile([128, 32], F32, tag="t31")
                V.tensor_tensor(t31[:], rz1[:, 32:64], t21[:], OP.mult)
                V.tensor_tensor(h1t[:].rearrange("p k b -> p (k b)"),
                                t31[:], n1[:], OP.add)
                S.activation(h1b[:].rearrange("p k b -> p (k b)"),
                             h1t[:].rearrange("p k b -> p (k b)"), AF.Copy)

                # --- output MLP ---
                for m in range(4):
                    for k in range(KH):
                        nc.tensor.matmul(py[:, m * BC:(m + 1) * BC],
                                         wot[:, k, m * 128:(m + 1) * 128],
                                         h1b[:, k, :],
                                         start=(m == 0 and k == 0),
                                         stop=(m == 3 and k == KH - 1))
                yb = wpool.tile([128, 32], F32, tag="yb")
                V.tensor_tensor(yb[:], py[:], bo1[:], OP.add)
                ytb = wpool.tile([128, KH, BC], BF16, tag="ytb")
                S.activation(ytb[:].rearrange("p k b -> p (k b)"), yb[:], AF.Relu)
                for k in range(KH):
                    nc.tensor.matmul(poa[:, :], ytb[:, k, :], wo2c[:, k:k + 1],
                                     start=(k == 0), stop=(k == KH - 1))
                for k in range(KH):
                    nc.tensor.matmul(pob[:, :], wo2r[:, k, :], ytb[:, k, :],
                                     start=(k == 0), stop=(k == KH - 1))
                S.activation(outsb[:, ds(iv, 1)], poa[:, :], AF.Relu,
                             bias=bo2r[0:BC, 0:1])
                S.activation(prevr[:], pob[:, :], AF.Relu, bias=bo2r[:, 0:1])

            with tc.For_i(0, n_steps, 1, staggered_reset=True) as iv:
                step(iv)

            # ---- store outputs ----
            nc.sync.dma_start(out=d_oy[:, :], in_=outsb[:])
            nc.sync.dma_start(out=d_oh[:, 0:32],
                              in_=h0t[:].rearrange("p k b -> p (k b)"))
            nc.sync.dma_start(out=d_oh[:, 32:64],
                              in_=h1t[:].rearrange("p k b -> p (k b)"))
    nc.compile()
    return nc


def _rt(x, kt):  # [kt*128, N] -> [128, kt*N] partition-tiled, row = idx % 128
    n = x.shape[1]
    return np.ascontiguousarray(
        x.reshape(kt, 128, n).transpose(1, 0, 2).reshape(128, kt * n))


def _bias_tile(v, nm):  # v: (nm*128,) -> [128, nm*8] replicated over batch
    t = v.reshape(nm, 128).T  # [128, nm]
    return np.ascontiguousarray(np.repeat(t[:, :, None], BC, axis=2)
                                .reshape(128, nm * BC))


def make_inputs_for_core(c, inputs, n_steps=T):
    f32 = np.float32
    bf = ml_dtypes.bfloat16
    s = slice(c * BC, (c + 1) * BC)
    ctx = np.asarray(inputs["context_vector"], f32)[s, 0, :]      # (8, 512)
    W_ih0 = np.asarray(inputs["W_ih0"], f32)
    W_hh0 = np.asarray(inputs["W_hh0"], f32)
    b_ih0 = np.asarray(inputs["b_ih0"], f32)
    b_hh0 = np.asarray(inputs["b_hh0"], f32)
    W_ih1 = np.asarray(inputs["W_ih1"], f32)
    W_hh1 = np.asarray(inputs["W_hh1"], f32)
    b_ih1 = np.asarray(inputs["b_ih1"], f32)
    b_hh1 = np.asarray(inputs["b_hh1"], f32)
    W_o1 = np.asarray(inputs["W_o1"], f32)
    b_o1 = np.asarray(inputs["b_o1"], f32)
    W_o2 = np.asarray(inputs["W_o2"], f32)
    b_o2 = np.asarray(inputs["b_o2"], f32)

    C0 = ctx @ W_ih0[:, :H].T + b_ih0               # (8, 1536)
    brz = b_hh0.copy()
    brz[2 * H:] = 0.0                                # b_hh0 only on r,z rows
    C0 = C0 + brz
    c0t = np.ascontiguousarray(
        C0.T.reshape(12, 128, BC).transpose(1, 0, 2).reshape(128, 96))
    wp = W_ih0[:, H]                                 # (1536,)
    wpt = _bias_tile(wp, 12)                         # [128, 96]

    w0t = _rt(W_hh0.T.astype(bf), KH)
    w1t = _rt(np.concatenate([W_ih1.T, W_hh1.T], 0).astype(bf), 2 * KH)
    wot = _rt(W_o1.T.astype(bf), KH)
    wo2c = _rt(W_o2.T.astype(bf), KH)                # [128, 4]
    wo2r = _rt(np.repeat(W_o2.T, 128, axis=1).astype(bf), KH)

    return {
        "w0t": w0t, "w1t": w1t, "wot": wot, "wo2c": wo2c, "wo2r": wo2r,
        "c0t": c0t.astype(f32), "wpt": wpt.astype(f32),
        "bhn0": _bias_tile(b_hh0[2 * H:], 4).astype(f32),
        "b1rz": _bias_tile((b_ih1 + b_hh1)[:2 * H], 8).astype(f32),
        "bin1": _bias_tile(b_ih1[2 * H:], 4).astype(f32),
        "bhn1": _bias_tile(b_hh1[2 * H:], 4).astype(f32),
        "bo1": _bias_tile(b_o1, 4).astype(f32),
        "bo2r": np.full((128, 1), b_o2[0], f32),
    }


def assemble_outputs(results, n_steps=T):
    """results: list per core of {'oy': (8, T), 'oh': (128, 64)}."""
    outputs = np.zeros((B, n_steps, 1), np.float32)
    h_i = np.zeros((2, B, H), np.float32)
    for c in range(NCORES):
        s = slice(c * BC, (c + 1) * BC)
        outputs[s, :, 0] = results[c]["oy"]
        oh = results[c]["oh"]
        h0 = oh[:, 0:32].reshape(128, KH, BC).transpose(2, 1, 0).reshape(BC, H)
        h1 = oh[:, 32:64].reshape(128, KH, BC).transpose(2, 1, 0).reshape(BC, H)
        h_i[0, s, :] = h0
        h_i[1, s, :] = h1
    return outputs, h_i


_NC_CACHE = {}


def kernel(**inputs):
    n_steps = int(np.asarray(inputs["expected_output"]).shape[1])
    key = n_steps
    if key not in _NC_CACHE:
        _NC_CACHE[key] = build_nc(n_steps)
    nc = _NC_CACHE[key]
    in_maps = [make_inputs_for_core(c, inputs, n_steps) for c in range(NCORES)]
    res = run_bass_kernel_spmd(nc, in_maps, list(range(NCORES)))
    return assemble_outputs(res.results, n_steps)


if __name__ == "__main__":
    import reference
    inputs = {k: np.asarray(v) for k, v in reference.setup_inputs().items()}
    out, h = kernel(**inputs)
    print(out.shape, h.shape)
